# revision 32
# baseline (speedup 1.0000x reference)
"""Two-layer GCN encoder on 8 Trainium2 NeuronCores (Bass/Tile).

Strategy (graph/data parallel, dst-sharded):
  - nodes sharded contiguously across 8 cores (6250 each, padded to 6272);
  - per core: ts = (x_c @ W1) * dinv rows  ->  AllGather -> full ts table;
  - edges (sorted by dst tile, split by src half for int16 dma_gather)
    gathered per 7-tile batch into SBUF message tiles M;
  - segment-sum as one-hot matmuls: psum[ch,dst] += M_chunk^T @ O_chunk,
    where O is built on-device via is_equal(dst_local, iota);
  - epilogue folds self-loop term, dinv scale, bias, relu;
  - layer 2 identical with t2s = (h1*dinv) @ W2 and 64 channels.
All float math runs on device; the host only does integer edge routing /
layout (sort, bucket, pad) and final unpermute.
"""
import sys
sys.path.insert(0, "/opt/trn_rl_repo")
import numpy as np
import ml_dtypes
import os
STOP = os.environ.get("K_STOP", "")
SUB = int(os.environ.get("K_SUB", "9"))

import concourse.bass as bass
import concourse.bacc as bacc
import concourse.mybir as mybir
from concourse.tile import TileContext
from concourse.masks import make_identity
from concourse.bass_utils import run_bass_kernel_spmd

DT = mybir.dt
LAST_EXEC_NS = None
LAST_RES = None
P = 128
NCORES = 8

# problem sizes (hardcoded per contest rules)
N_NODES = 50000
IN_CH = 256
HID = 128
OUT = 64

NPC = N_NODES // NCORES          # 6250 nodes per core
NT = (NPC + P - 1) // P          # 49 tiles per core
NPAD = NT * P                    # 6272 padded nodes per core
NFULL = NPAD * NCORES            # 50176 padded table rows
HALF = NFULL // 2                # 25088 split for int16 gather indices
TB = 2                           # max tiles per gather batch
NB = NT // TB
BATCH_SIZES = [2] * 24 + [1]


def _balance(dst, src):
    """Permute each core's nodes across its 49 tiles so per-(tile,half)
    edge counts stay under 9*128, minimizing gather padding. Returns
    perm (new local position for each node) - integer layout only."""
    deg_lo = np.bincount(dst[src < N_NODES // 2], minlength=N_NODES)
    deg_hi = np.bincount(dst[src >= N_NODES // 2], minlength=N_NODES)
    perm = np.empty(N_NODES, dtype=np.int64)
    cap = 9 * P
    for c in range(NCORES):
        lo = deg_lo[c * NPC:(c + 1) * NPC]
        hi = deg_hi[c * NPC:(c + 1) * NPC]
        order = np.argsort(-(lo + hi), kind="stable")
        t_lo = np.zeros(NT, dtype=np.int64)
        t_hi = np.zeros(NT, dtype=np.int64)
        t_cnt = np.zeros(NT, dtype=np.int64)
        slot = np.empty(NPC, dtype=np.int64)
        for n in order:
            # first feasible tile by current max load (greedy best-fit)
            t_best = -1
            best = 1 << 40
            for t in range(NT):
                if t_cnt[t] >= P or (t == NT - 1 and t_cnt[t] >= NPC - (NT - 1) * P):
                    continue
                if t_lo[t] + lo[n] > cap or t_hi[t] + hi[n] > cap:
                    continue
                load = max(t_lo[t] + lo[n], t_hi[t] + hi[n])
                if load < best:
                    best = load
                    t_best = t
            if t_best < 0:  # fall back: least-loaded non-full tile
                t_best = int(np.argmin(np.where(t_cnt < P, t_lo + t_hi, 1 << 40)))
            slot[n] = t_best * P + t_cnt[t_best]
            t_cnt[t_best] += 1
            t_lo[t_best] += lo[n]
            t_hi[t_best] += hi[n]
        perm[c * NPC:(c + 1) * NPC] = slot
    return perm


def _preprocess(edge_index):
    """Integer-only edge routing. Returns per-core index/layout arrays."""
    src = np.asarray(edge_index[0], dtype=np.int64)
    dst = np.asarray(edge_index[1], dtype=np.int64)

    deg = np.bincount(dst, minlength=N_NODES) + 1  # + self loop

    perm = _balance(dst, src)   # new local slot of node n within its core

    core = dst // NPC
    local = perm[dst]           # balanced local position (0..NPC-1, <NT*P)
    tile = local >> 7
    dloc = local & 127
    half = (src >= N_NODES // 2).astype(np.int64)
    gtile = core * NT + tile                  # 0..391
    bucket = gtile * 2 + half                 # 0..783

    # secondary sort by src for HBM row locality inside each gather
    order = np.lexsort((src, bucket))
    b_sorted = bucket[order]
    src_s = src[order]
    dloc_s = dloc[order]

    counts = np.bincount(bucket, minlength=NT * NCORES * 2)
    KLO = int(np.ceil(counts[0::2].max() / P))
    KHI = int(np.ceil(counts[1::2].max() / P))

    starts = np.zeros(NT * NCORES * 2, dtype=np.int64)
    starts[1:] = np.cumsum(counts)[:-1]
    pos_in_bucket = np.arange(len(order)) - starts[b_sorted]

    # padded source id in table space (permuted local position)
    gid = (src_s // NPC) * NPAD + perm[src_s]
    idx_lo_val = gid                      # < HALF for half==0
    idx_hi_val = gid - HALF

    cores_dat = []
    for c in range(NCORES):
        lo_stream = np.zeros(NT * KLO * P, dtype=np.int16)
        hi_stream = np.zeros(NT * KHI * P, dtype=np.int16)
        dloc_arr = np.full((P, NT * (KLO + KHI)), 255.0, dtype=np.float32)

        m = (b_sorted >> 1) // NT == c
        bs = b_sorted[m]
        t_loc = (bs >> 1) % NT
        is_hi = bs & 1
        pos = pos_in_bucket[m]
        sv = src_s[m]
        dl = dloc_s[m]
        gl = gid[m]

        mlo = is_hi == 0
        i_lo = t_loc[mlo] * (KLO * P) + pos[mlo]
        lo_stream[i_lo] = gl[mlo].astype(np.int16)
        dloc_arr[pos[mlo] & 127, t_loc[mlo] * (KLO + KHI) + (pos[mlo] >> 7)] = dl[mlo]

        mhi = is_hi == 1
        i_hi = t_loc[mhi] * (KHI * P) + pos[mhi]
        hi_stream[i_hi] = (gl[mhi] - HALF).astype(np.int16)
        dloc_arr[pos[mhi] & 127,
                 t_loc[mhi] * (KLO + KHI) + KLO + (pos[mhi] >> 7)] = dl[mhi]

        def pack16(flat):
            # idx i -> (partition i%16, col i//16), replicated to 8 groups
            a = flat.reshape(-1, 16).T
            return np.ascontiguousarray(np.tile(a, (8, 1)))

        deg_c = np.ones((P, NT), dtype=np.int32)
        pl = perm[c * NPC:(c + 1) * NPC]
        deg_c[pl & 127, pl >> 7] = deg[c * NPC:(c + 1) * NPC]

        cores_dat.append({
            "idx_lo": pack16(lo_stream),
            "idx_hi": pack16(hi_stream),
            "dloc": dloc_arr,
            "deg": deg_c,
        })
    return cores_dat, KLO, KHI, perm


def _build_program(KLO, KHI):
    KT = KLO + KHI                 # chunks per tile (one-hot columns)
    CB_LO = TB * KLO               # lo chunks per batch
    CB_HI = TB * KHI
    CB = CB_LO + CB_HI             # chunks per batch in M
    nc = bacc.Bacc("TRN2", target_bir_lowering=False, num_devices=NCORES,
                   num_swdge_queues=4)

    x_in = nc.dram_tensor("x_t", [IN_CH, NPAD], DT.float32, kind="ExternalInput")
    w1_in = nc.dram_tensor("w1", [IN_CH, HID], DT.float32, kind="ExternalInput")
    b1_in = nc.dram_tensor("b1", [HID, 1], DT.float32, kind="ExternalInput")
    w2_in = nc.dram_tensor("w2", [HID, OUT], DT.float32, kind="ExternalInput")
    b2_in = nc.dram_tensor("b2", [OUT, 1], DT.float32, kind="ExternalInput")
    deg_in = nc.dram_tensor("deg", [P, NT], DT.int32, kind="ExternalInput")
    ilo_in = nc.dram_tensor("idx_lo", [P, NT * KLO * 8], DT.int16, kind="ExternalInput")
    ihi_in = nc.dram_tensor("idx_hi", [P, NT * KHI * 8], DT.int16, kind="ExternalInput")
    dloc_in = nc.dram_tensor("dloc", [P, NT * KT], DT.float32, kind="ExternalInput")
    out_t = nc.dram_tensor("out_t", [OUT, NPAD], DT.float32, kind="ExternalOutput")

    ts_shard = nc.dram_tensor("ts_shard", [NPAD, HID], DT.bfloat16)
    ts_full = nc.dram_tensor("ts_full", [NFULL, HID], DT.bfloat16, addr_space="Shared")
    t2_shard = nc.dram_tensor("t2_shard", [NPAD, HID], DT.bfloat16)
    t2_full = nc.dram_tensor("t2_full", [NFULL, HID], DT.bfloat16, addr_space="Shared")

    with TileContext(nc) as tc:
        with (
            tc.tile_pool(name="const", bufs=1) as cp,
            tc.tile_pool(name="xk", bufs=4) as xp,
            tc.tile_pool(name="work", bufs=4) as wp,
            tc.tile_pool(name="mbuf", bufs=7) as mp,
            tc.tile_pool(name="obuf", bufs=4) as op,
            tc.tile_pool(name="psA", bufs=2, space="PSUM") as psA,
            tc.tile_pool(name="psB", bufs=2, space="PSUM") as psB,
            tc.tile_pool(name="psC", bufs=2, space="PSUM") as psC,
        ):
            # ---------- phase A: constants ----------
            idx_lo = cp.tile([P, NT * KLO * 8], DT.int16)
            nc.sync.dma_start(out=idx_lo[:], in_=ilo_in[:])
            idx_hi = cp.tile([P, NT * KHI * 8], DT.int16)
            nc.sync.dma_start(out=idx_hi[:], in_=ihi_in[:])
            dloc_f = cp.tile([P, NT * KT], DT.float32)
            nc.sync.dma_start(out=dloc_f[:], in_=dloc_in[:])
            dloc = cp.tile([P, NT * KT], DT.bfloat16)
            nc.vector.tensor_copy(out=dloc[:], in_=dloc_f[:])

            w1a = cp.tile([P, HID], DT.float32)
            nc.sync.dma_start(out=w1a[:], in_=w1_in[0:P, :])
            w1b = cp.tile([P, HID], DT.float32)
            nc.sync.dma_start(out=w1b[:], in_=w1_in[P:2 * P, :])
            w2f = cp.tile([HID, OUT], DT.float32)
            nc.sync.dma_start(out=w2f[:], in_=w2_in[:])
            w2 = cp.tile([HID, OUT], DT.bfloat16)
            nc.vector.tensor_copy(out=w2[:], in_=w2f[:])
            b1t = cp.tile([HID, 1], DT.float32)
            nc.sync.dma_start(out=b1t[:], in_=b1_in[:])
            b2t = cp.tile([OUT, 1], DT.float32)
            nc.sync.dma_start(out=b2t[:], in_=b2_in[:])

            deg_i = cp.tile([P, NT], DT.int32)
            nc.sync.dma_start(out=deg_i[:], in_=deg_in[:])
            deg_f = cp.tile([P, NT], DT.float32)
            nc.vector.tensor_copy(out=deg_f[:], in_=deg_i[:])
            drec = cp.tile([P, NT], DT.float32)
            nc.vector.reciprocal(drec[:], deg_f[:])
            dinv = cp.tile([P, NT], DT.float32)
            nc.scalar.activation(dinv[:], drec[:], mybir.ActivationFunctionType.Sqrt)

            iota_i = cp.tile([P, P], DT.int32)
            nc.gpsimd.iota(iota_i[:], pattern=[[1, P]], base=0, channel_multiplier=0)
            iota_bf = cp.tile([P, P], DT.bfloat16)
            nc.vector.tensor_copy(out=iota_bf[:], in_=iota_i[:])

            identf = cp.tile([P, P], DT.float32)
            make_identity(nc, identf[:])
            ones = cp.tile([P, P], DT.float32)
            nc.gpsimd.memset(ones[:], 1.0)

            # dinv in column-broadcast layout: dinv_cols[:, t*128+j] = dinv[j, t]
            dinv_cols = cp.tile([P, NPAD], DT.float32)
            tsT_own = cp.tile([P, NPAD], DT.float32)
            hsT = cp.tile([P, NPAD], DT.bfloat16)
            t2T_own = cp.tile([OUT, NPAD], DT.float32)

            # ---------- phase B: GEMM1 both orientations ----------
            if STOP == "A":
                nc.sync.dma_start(out=out_t[:, 0:P], in_=dinv_cols[0:OUT, 0:P])

            for t in range(NT if (not STOP or STOP >= "B") else 0):
                xk0 = xp.tile([P, P], DT.float32, tag="xk0")
                nc.sync.dma_start(out=xk0[:], in_=x_in[0:P, t * P:(t + 1) * P])
                xk1 = xp.tile([P, P], DT.float32, tag="xk1")
                nc.sync.dma_start(out=xk1[:], in_=x_in[P:2 * P, t * P:(t + 1) * P])
                # node-major: psum[node, ch] = x_tile @ W1
                pn = psA.tile([P, HID], DT.float32, space="PSUM", tag="m1")
                nc.tensor.matmul(out=pn[:], lhsT=xk0[:], rhs=w1a[:],
                                 start=True, stop=False)
                nc.tensor.matmul(out=pn[:], lhsT=xk1[:], rhs=w1b[:],
                                 start=False, stop=True)
                ts_bf = wp.tile([P, HID], DT.bfloat16, tag="tsbf")
                nc.vector.tensor_scalar(
                    out=ts_bf[:], in0=pn[:], scalar1=dinv[:, t:t + 1],
                    scalar2=None, op0=mybir.AluOpType.mult)
                nc.sync.dma_start(out=ts_shard[t * P:(t + 1) * P, :], in_=ts_bf[:])

            if STOP == "B":
                nc.sync.dma_start(out=out_t[:, 0:P], in_=dinv_cols[0:OUT, 0:P])
            # ---------- phase C: allgather ts ----------
            if not STOP or STOP >= "C":
                nc.gpsimd.collective_compute(
                    "AllGather", mybir.AluOpType.bypass,
                    replica_groups=[list(range(NCORES))],
                    ins=[ts_shard[:]], outs=[ts_full[:]])

            # ---------- phase B2 (overlaps AG1): dinv_cols + tsT_own ----------
            for t in range(NT if (not STOP or STOP >= "B") else 0):
                diag = wp.tile([P, P], DT.float32, tag="diag")
                nc.vector.tensor_scalar(
                    out=diag[:], in0=identf[:], scalar1=dinv[:, t:t + 1],
                    scalar2=None, op0=mybir.AluOpType.mult)
                dps = psA.tile([P, P], DT.float32, space="PSUM", tag="m1")
                nc.tensor.matmul(out=dps[:], lhsT=ones[:], rhs=diag[:],
                                 start=True, stop=True)
                nc.vector.tensor_copy(out=dinv_cols[:, t * P:(t + 1) * P], in_=dps[:])
            for t in range(NT if (not STOP or STOP >= "B") else 0):
                xk0 = xp.tile([P, P], DT.float32, tag="xk0")
                nc.sync.dma_start(out=xk0[:], in_=x_in[0:P, t * P:(t + 1) * P])
                xk1 = xp.tile([P, P], DT.float32, tag="xk1")
                nc.sync.dma_start(out=xk1[:], in_=x_in[P:2 * P, t * P:(t + 1) * P])
                pt = psB.tile([HID, P], DT.float32, space="PSUM", tag="m2")
                nc.tensor.matmul(out=pt[:], lhsT=w1a[:], rhs=xk0[:],
                                 start=True, stop=False)
                nc.tensor.matmul(out=pt[:], lhsT=w1b[:], rhs=xk1[:],
                                 start=False, stop=True)
                nc.vector.tensor_tensor(
                    out=tsT_own[:, t * P:(t + 1) * P], in0=pt[:],
                    in1=dinv_cols[:, t * P:(t + 1) * P], op=mybir.AluOpType.mult)

            if STOP == "C":
                nc.sync.dma_start(out=out_t[:, 0:P], in_=dinv_cols[0:OUT, 0:P])
            # ---------- phase D/G: aggregation layers ----------
            def agg_layer(src_full, selfT, fch, bias, post, dst_sink):
                """fch: message channels used (HID or OUT); selfT [fch, NPAD];
                post(tile_idx, acc_sbuf_tile) emits per-tile outputs."""
                GW = 8   # chunks per gather instruction (<=1024 idxs)
                qrot = [0]
                tile_batches = []
                t0 = 0
                for bs in BATCH_SIZES:
                    tile_batches.append(list(range(t0, t0 + bs)))
                    t0 += bs
                for b, tiles in enumerate(tile_batches):
                    TBv = len(tiles)
                    CB_LOv = TBv * KLO
                    CB_HIv = TBv * KHI
                    M = mp.tile([P, CB * HID], DT.bfloat16, tag="M")
                    blo0 = tiles[0] * KLO
                    bhi0 = tiles[0] * KHI
                    for g0 in range(0, CB_LOv, GW):
                        w = min(GW, CB_LOv - g0)
                        n = w * P
                        col0 = (blo0 + g0) * 8
                        nc.gpsimd.dma_gather(
                            out_ap=M[:, g0 * HID:(g0 + w) * HID].rearrange(
                                "p (c f) -> p c f", f=HID),
                            in_ap=src_full[0:HALF, :],
                            idxs_ap=idx_lo[:, col0:col0 + w * 8],
                            num_idxs=n, num_idxs_reg=n, elem_size=HID,
                            queue_num=qrot[0] % 4)
                        qrot[0] += 1
                    for g0 in range(0, CB_HIv, GW):
                        w = min(GW, CB_HIv - g0)
                        n = w * P
                        col0 = (bhi0 + g0) * 8
                        nc.gpsimd.dma_gather(
                            out_ap=M[:, (CB_LOv + g0) * HID:(CB_LOv + g0 + w) * HID
                                     ].rearrange("p (c f) -> p c f", f=HID),
                            in_ap=src_full[HALF:NFULL, :],
                            idxs_ap=idx_hi[:, col0:col0 + w * 8],
                            num_idxs=n, num_idxs_reg=n, elem_size=HID,
                            queue_num=qrot[0] % 4)
                        qrot[0] += 1
                    for ti in range(TBv if SUB >= 2 else 0):
                        t = tiles[ti]
                        O = op.tile([P, KT * P], DT.bfloat16, tag="O")
                        ia = iota_bf[:]
                        iota_mid = bass.AP(
                            ia.tensor, ia.offset, [ia.ap[0], [0, KT], ia.ap[1]])
                        nc.vector.tensor_tensor(
                            out=O[:].rearrange("p (c j) -> p c j", j=P),
                            in0=dloc[:, t * KT:(t + 1) * KT].to_broadcast([P, KT, P]),
                            in1=iota_mid, op=mybir.AluOpType.is_equal)
                        if SUB < 3:
                            continue
                        acc = psC.tile([fch, P], DT.float32, space="PSUM", tag="acc")
                        for k in range(KLO):
                            c = (CB_LO - CB_LOv) // KLO * 0 + ti * KLO + k
                            nc.tensor.matmul(
                                out=acc[:],
                                lhsT=M[:, c * HID:c * HID + fch],
                                rhs=O[:, k * P:(k + 1) * P],
                                start=(k == 0), stop=False)
                        for k in range(KHI):
                            c = CB_LOv + ti * KHI + k
                            nc.tensor.matmul(
                                out=acc[:],
                                lhsT=M[:, c * HID:c * HID + fch],
                                rhs=O[:, (KLO + k) * P:(KLO + k + 1) * P],
                                start=False, stop=(k == KHI - 1))
                        # epilogue: (acc + selfT_t) * dinv_cols_t (+bias, act)
                        if SUB < 4:
                            z = wp.tile([fch, P], DT.float32, tag="a1")
                            nc.vector.tensor_copy(out=z[:], in_=acc[:])
                            continue
                        a1 = wp.tile([fch, P], DT.float32, tag="a1")
                        nc.vector.tensor_tensor(
                            out=a1[:], in0=acc[:],
                            in1=selfT[:, t * P:(t + 1) * P],
                            op=mybir.AluOpType.add)
                        nc.vector.tensor_tensor(
                            out=a1[:], in0=a1[:],
                            in1=dinv_cols[0:fch, t * P:(t + 1) * P],
                            op=mybir.AluOpType.mult)
                        post(t, a1, bias)

            def post_l1(t, a1, bias):
                h = wp.tile([HID, P], DT.float32, tag="h")
                nc.scalar.activation(h[:], a1[:],
                                     mybir.ActivationFunctionType.Relu,
                                     bias=bias[:], scale=1.0)
                nc.vector.tensor_tensor(
                    out=hsT[:, t * P:(t + 1) * P], in0=h[:],
                    in1=dinv_cols[:, t * P:(t + 1) * P],
                    op=mybir.AluOpType.mult)
                # GEMM2 for this tile immediately (feeds AG2 asap)
                p2 = psA.tile([P, OUT], DT.float32, space="PSUM", tag="m1")
                nc.tensor.matmul(out=p2[:], lhsT=hsT[:, t * P:(t + 1) * P],
                                 rhs=w2[:], start=True, stop=True)
                t2_bf = wp.tile([P, OUT], DT.bfloat16, tag="t2bf")
                nc.vector.tensor_copy(out=t2_bf[:], in_=p2[:])
                nc.sync.dma_start(out=t2_shard[t * P:(t + 1) * P, 0:OUT],
                                  in_=t2_bf[:])
                p2t = psB.tile([OUT, P], DT.float32, space="PSUM", tag="m2")
                nc.tensor.matmul(out=p2t[:], lhsT=w2[:],
                                 rhs=hsT[:, t * P:(t + 1) * P],
                                 start=True, stop=True)
                nc.vector.tensor_copy(out=t2T_own[:, t * P:(t + 1) * P], in_=p2t[:])

            if not STOP or STOP >= "D":
                agg_layer(ts_full, tsT_own, HID, b1t, post_l1, None)
            if STOP == "D":
                if SUB >= 4:
                    nc.gpsimd.dma_start(out=out_t[:, 0:P], in_=hsT[0:OUT, 0:P])
                else:
                    nc.sync.dma_start(out=out_t[:, 0:P], in_=dinv_cols[0:OUT, 0:P])

            if STOP == "E":
                nc.sync.dma_start(out=out_t[:, 0:P], in_=t2T_own[:, 0:P])
            # ---------- phase F: allgather t2 ----------
            if STOP and STOP < "F":
                pass
            else:
                nc.gpsimd.collective_compute(
                "AllGather", mybir.AluOpType.bypass,
                replica_groups=[list(range(NCORES))],
                ins=[t2_shard[:]], outs=[t2_full[:]])

            def post_l2(t, a1, bias):
                o = wp.tile([OUT, P], DT.float32, tag="o")
                nc.vector.tensor_scalar(
                    out=o[:], in0=a1[:], scalar1=bias[:], scalar2=None,
                    op0=mybir.AluOpType.add)
                nc.sync.dma_start(out=out_t[:, t * P:(t + 1) * P], in_=o[:])

            if not STOP:
                agg_layer(t2_full, t2T_own, OUT, b2t, post_l2, None)

    nc.compile()
    return nc


def kernel(x, edge_index, W1, b1, W2, b2):
    x = np.asarray(x, dtype=np.float32)
    W1 = np.asarray(W1, dtype=np.float32)
    b1 = np.asarray(b1, dtype=np.float32)
    W2 = np.asarray(W2, dtype=np.float32)
    b2 = np.asarray(b2, dtype=np.float32)

    import time as _t
    _t0 = _t.time()
    cores_dat, KLO, KHI, perm = _preprocess(edge_index)
    print(f"[kernel] preprocess {_t.time()-_t0:.1f}s KLO={KLO} KHI={KHI}", file=sys.stderr)
    _t0 = _t.time()
    nc = _build_program(KLO, KHI)
    print(f"[kernel] build+bacc-compile {_t.time()-_t0:.1f}s", file=sys.stderr)

    in_maps = []
    for c in range(NCORES):
        xt = np.zeros((IN_CH, NPAD), dtype=np.float32)
        xt[:, perm[c * NPC:(c + 1) * NPC]] = x[c * NPC:(c + 1) * NPC].T
        d = cores_dat[c]
        in_maps.append({
            "x_t": xt,
            "w1": W1,
            "b1": b1.reshape(HID, 1),
            "w2": W2,
            "b2": b2.reshape(OUT, 1),
            "deg": d["deg"],
            "idx_lo": d["idx_lo"],
            "idx_hi": d["idx_hi"],
            "dloc": d["dloc"],
        })
    _t0 = _t.time()
    trace = bool(os.environ.get("K_TRACE"))
    res = run_bass_kernel_spmd(nc, in_maps, list(range(NCORES)), trace=trace)
    print(f"[kernel] run {_t.time()-_t0:.1f}s", file=sys.stderr)
    global LAST_EXEC_NS, LAST_RES
    LAST_EXEC_NS = res.exec_time_ns
    LAST_RES = res
    if trace:
        print(f"[kernel] exec_time_ns={res.exec_time_ns}", file=sys.stderr)
    out = np.empty((N_NODES, OUT), dtype=np.float32)
    for c in range(NCORES):
        out[c * NPC:(c + 1) * NPC] = \
            res.results[c]["out_t"].T[perm[c * NPC:(c + 1) * NPC]]
    return out


# revision 34
# speedup vs baseline: 1.0256x; 1.0256x over previous
"""Two-layer GCN encoder on 8 Trainium2 NeuronCores (Bass/Tile).

Strategy (graph/data parallel, dst-sharded):
  - nodes sharded contiguously across 8 cores (6250 each, padded to 6272);
  - per core: ts = (x_c @ W1) * dinv rows  ->  AllGather -> full ts table;
  - edges (sorted by dst tile, split by src half for int16 dma_gather)
    gathered per 7-tile batch into SBUF message tiles M;
  - segment-sum as one-hot matmuls: psum[ch,dst] += M_chunk^T @ O_chunk,
    where O is built on-device via is_equal(dst_local, iota);
  - epilogue folds self-loop term, dinv scale, bias, relu;
  - layer 2 identical with t2s = (h1*dinv) @ W2 and 64 channels.
All float math runs on device; the host only does integer edge routing /
layout (sort, bucket, pad) and final unpermute.
"""
import sys
sys.path.insert(0, "/opt/trn_rl_repo")
import numpy as np
import ml_dtypes
import os
STOP = os.environ.get("K_STOP", "")
SUB = int(os.environ.get("K_SUB", "9"))

import concourse.bass as bass
import concourse.bacc as bacc
import concourse.mybir as mybir
from concourse.tile import TileContext
from concourse.masks import make_identity
from concourse.bass_utils import run_bass_kernel_spmd

DT = mybir.dt
LAST_EXEC_NS = None
LAST_RES = None
P = 128
NCORES = 8

# problem sizes (hardcoded per contest rules)
N_NODES = 50000
IN_CH = 256
HID = 128
OUT = 64

NPC = N_NODES // NCORES          # 6250 nodes per core
NT = (NPC + P - 1) // P          # 49 tiles per core
NPAD = NT * P                    # 6272 padded nodes per core
NFULL = NPAD * NCORES            # 50176 padded table rows
HALF = NFULL // 2                # 25088 split for int16 gather indices
TB = 2                           # max tiles per gather batch
NB = NT // TB
BATCH_SIZES = [2] * 24 + [1]


def _balance(dst, src):
    """Permute each core's nodes across its 49 tiles so per-(tile,half)
    edge counts stay under 9*128, minimizing gather padding. Returns
    perm (new local position for each node) - integer layout only."""
    deg_lo = np.bincount(dst[src < N_NODES // 2], minlength=N_NODES)
    deg_hi = np.bincount(dst[src >= N_NODES // 2], minlength=N_NODES)
    perm = np.empty(N_NODES, dtype=np.int64)
    cap = 9 * P
    for c in range(NCORES):
        lo = deg_lo[c * NPC:(c + 1) * NPC]
        hi = deg_hi[c * NPC:(c + 1) * NPC]
        order = np.argsort(-(lo + hi), kind="stable")
        t_lo = np.zeros(NT, dtype=np.int64)
        t_hi = np.zeros(NT, dtype=np.int64)
        t_cnt = np.zeros(NT, dtype=np.int64)
        slot = np.empty(NPC, dtype=np.int64)
        for n in order:
            # first feasible tile by current max load (greedy best-fit)
            t_best = -1
            best = 1 << 40
            for t in range(NT):
                if t_cnt[t] >= P or (t == NT - 1 and t_cnt[t] >= NPC - (NT - 1) * P):
                    continue
                if t_lo[t] + lo[n] > cap or t_hi[t] + hi[n] > cap:
                    continue
                load = max(t_lo[t] + lo[n], t_hi[t] + hi[n])
                if load < best:
                    best = load
                    t_best = t
            if t_best < 0:  # fall back: least-loaded non-full tile
                t_best = int(np.argmin(np.where(t_cnt < P, t_lo + t_hi, 1 << 40)))
            slot[n] = t_best * P + t_cnt[t_best]
            t_cnt[t_best] += 1
            t_lo[t_best] += lo[n]
            t_hi[t_best] += hi[n]
        perm[c * NPC:(c + 1) * NPC] = slot
    return perm


def _preprocess(edge_index):
    """Integer-only edge routing. Returns per-core index/layout arrays."""
    src = np.asarray(edge_index[0], dtype=np.int64)
    dst = np.asarray(edge_index[1], dtype=np.int64)

    deg = np.bincount(dst, minlength=N_NODES) + 1  # + self loop

    perm = _balance(dst, src)   # new local slot of node n within its core

    core = dst // NPC
    local = perm[dst]           # balanced local position (0..NPC-1, <NT*P)
    tile = local >> 7
    dloc = local & 127
    half = (src >= N_NODES // 2).astype(np.int64)
    gtile = core * NT + tile                  # 0..391
    bucket = gtile * 2 + half                 # 0..783

    # secondary sort by src for HBM row locality inside each gather
    order = np.lexsort((src, bucket))
    b_sorted = bucket[order]
    src_s = src[order]
    dloc_s = dloc[order]

    counts = np.bincount(bucket, minlength=NT * NCORES * 2)
    KLO = int(np.ceil(counts[0::2].max() / P))
    KHI = int(np.ceil(counts[1::2].max() / P))

    starts = np.zeros(NT * NCORES * 2, dtype=np.int64)
    starts[1:] = np.cumsum(counts)[:-1]
    pos_in_bucket = np.arange(len(order)) - starts[b_sorted]

    # padded source id in table space (permuted local position)
    gid = (src_s // NPC) * NPAD + perm[src_s]
    idx_lo_val = gid                      # < HALF for half==0
    idx_hi_val = gid - HALF

    cores_dat = []
    for c in range(NCORES):
        lo_stream = np.zeros(NT * KLO * P, dtype=np.int16)
        hi_stream = np.zeros(NT * KHI * P, dtype=np.int16)
        dloc_arr = np.full((P, NT * (KLO + KHI)), 255.0, dtype=np.float32)

        m = (b_sorted >> 1) // NT == c
        bs = b_sorted[m]
        t_loc = (bs >> 1) % NT
        is_hi = bs & 1
        pos = pos_in_bucket[m]
        sv = src_s[m]
        dl = dloc_s[m]
        gl = gid[m]

        mlo = is_hi == 0
        i_lo = t_loc[mlo] * (KLO * P) + pos[mlo]
        lo_stream[i_lo] = gl[mlo].astype(np.int16)
        dloc_arr[pos[mlo] & 127, t_loc[mlo] * (KLO + KHI) + (pos[mlo] >> 7)] = dl[mlo]

        mhi = is_hi == 1
        i_hi = t_loc[mhi] * (KHI * P) + pos[mhi]
        hi_stream[i_hi] = (gl[mhi] - HALF).astype(np.int16)
        dloc_arr[pos[mhi] & 127,
                 t_loc[mhi] * (KLO + KHI) + KLO + (pos[mhi] >> 7)] = dl[mhi]

        def pack16(flat):
            # idx i -> (partition i%16, col i//16), replicated to 8 groups
            a = flat.reshape(-1, 16).T
            return np.ascontiguousarray(np.tile(a, (8, 1)))

        deg_c = np.ones((P, NT), dtype=np.int32)
        pl = perm[c * NPC:(c + 1) * NPC]
        deg_c[pl & 127, pl >> 7] = deg[c * NPC:(c + 1) * NPC]

        cores_dat.append({
            "idx_lo": pack16(lo_stream),
            "idx_hi": pack16(hi_stream),
            "dloc": dloc_arr,
            "deg": deg_c,
        })
    return cores_dat, KLO, KHI, perm


def _build_program(KLO, KHI):
    KT = KLO + KHI                 # chunks per tile (one-hot columns)
    CB_LO = TB * KLO               # lo chunks per batch
    CB_HI = TB * KHI
    CB = CB_LO + CB_HI             # chunks per batch in M
    nc = bacc.Bacc("TRN2", target_bir_lowering=False, num_devices=NCORES,
                   num_swdge_queues=4)

    x_in = nc.dram_tensor("x_t", [IN_CH, NPAD], DT.float32, kind="ExternalInput")
    w1_in = nc.dram_tensor("w1", [IN_CH, HID], DT.float32, kind="ExternalInput")
    b1_in = nc.dram_tensor("b1", [HID, 1], DT.float32, kind="ExternalInput")
    w2_in = nc.dram_tensor("w2", [HID, OUT], DT.float32, kind="ExternalInput")
    b2_in = nc.dram_tensor("b2", [OUT, 1], DT.float32, kind="ExternalInput")
    deg_in = nc.dram_tensor("deg", [P, NT], DT.int32, kind="ExternalInput")
    ilo_in = nc.dram_tensor("idx_lo", [P, NT * KLO * 8], DT.int16, kind="ExternalInput")
    ihi_in = nc.dram_tensor("idx_hi", [P, NT * KHI * 8], DT.int16, kind="ExternalInput")
    dloc_in = nc.dram_tensor("dloc", [P, NT * KT], DT.float32, kind="ExternalInput")
    out_t = nc.dram_tensor("out_t", [OUT, NPAD], DT.float32, kind="ExternalOutput")

    ts_shard = nc.dram_tensor("ts_shard", [NPAD, HID], DT.bfloat16)
    ts_full = nc.dram_tensor("ts_full", [NFULL, HID], DT.bfloat16, addr_space="Shared")
    t2_shard = nc.dram_tensor("t2_shard", [NPAD, HID], DT.bfloat16)
    t2_full = nc.dram_tensor("t2_full", [NFULL, HID], DT.bfloat16, addr_space="Shared")

    with TileContext(nc) as tc:
        with (
            tc.tile_pool(name="const", bufs=1) as cp,
            tc.tile_pool(name="xk", bufs=3) as xp,
            tc.tile_pool(name="work", bufs=3) as wp,
            tc.tile_pool(name="mbuf", bufs=6) as mp,
            tc.tile_pool(name="obuf", bufs=3) as op,
            tc.tile_pool(name="psA", bufs=2, space="PSUM") as psA,
            tc.tile_pool(name="psB", bufs=2, space="PSUM") as psB,
            tc.tile_pool(name="psC", bufs=3, space="PSUM") as psC,
        ):
            # ---------- phase A: constants ----------
            idx_lo = cp.tile([P, NT * KLO * 8], DT.int16)
            nc.sync.dma_start(out=idx_lo[:], in_=ilo_in[:])
            idx_hi = cp.tile([P, NT * KHI * 8], DT.int16)
            nc.sync.dma_start(out=idx_hi[:], in_=ihi_in[:])
            dloc_f = cp.tile([P, NT * KT], DT.float32)
            nc.sync.dma_start(out=dloc_f[:], in_=dloc_in[:])
            dloc = cp.tile([P, NT * KT], DT.bfloat16)
            nc.vector.tensor_copy(out=dloc[:], in_=dloc_f[:])

            w1a = cp.tile([P, HID], DT.float32)
            nc.sync.dma_start(out=w1a[:], in_=w1_in[0:P, :])
            w1b = cp.tile([P, HID], DT.float32)
            nc.sync.dma_start(out=w1b[:], in_=w1_in[P:2 * P, :])
            w2f = cp.tile([HID, OUT], DT.float32)
            nc.sync.dma_start(out=w2f[:], in_=w2_in[:])
            w2 = cp.tile([HID, OUT], DT.bfloat16)
            nc.vector.tensor_copy(out=w2[:], in_=w2f[:])
            b1t = cp.tile([HID, 1], DT.float32)
            nc.sync.dma_start(out=b1t[:], in_=b1_in[:])
            b2t = cp.tile([OUT, 1], DT.float32)
            nc.sync.dma_start(out=b2t[:], in_=b2_in[:])

            deg_i = cp.tile([P, NT], DT.int32)
            nc.sync.dma_start(out=deg_i[:], in_=deg_in[:])
            deg_f = cp.tile([P, NT], DT.float32)
            nc.vector.tensor_copy(out=deg_f[:], in_=deg_i[:])
            drec = cp.tile([P, NT], DT.float32)
            nc.vector.reciprocal(drec[:], deg_f[:])
            dinv = cp.tile([P, NT], DT.float32)
            nc.scalar.activation(dinv[:], drec[:], mybir.ActivationFunctionType.Sqrt)

            iota_i = cp.tile([P, P], DT.int32)
            nc.gpsimd.iota(iota_i[:], pattern=[[1, P]], base=0, channel_multiplier=0)
            iota_bf = cp.tile([P, P], DT.bfloat16)
            nc.vector.tensor_copy(out=iota_bf[:], in_=iota_i[:])

            identf = cp.tile([P, P], DT.float32)
            make_identity(nc, identf[:])
            ones = cp.tile([P, P], DT.float32)
            nc.gpsimd.memset(ones[:], 1.0)

            # dinv in column-broadcast layout: dinv_cols[:, t*128+j] = dinv[j, t]
            dinv_cols = cp.tile([P, NPAD], DT.float32)
            tsT_own = cp.tile([P, NPAD], DT.float32)
            hsT = cp.tile([P, NPAD], DT.bfloat16)
            t2T_own = cp.tile([OUT, NPAD], DT.float32)

            # ---------- phase B: GEMM1 both orientations ----------
            if STOP == "A":
                nc.sync.dma_start(out=out_t[:, 0:P], in_=dinv_cols[0:OUT, 0:P])

            for t in range(NT if (not STOP or STOP >= "B") else 0):
                xk0 = xp.tile([P, P], DT.float32, tag="xk0")
                nc.sync.dma_start(out=xk0[:], in_=x_in[0:P, t * P:(t + 1) * P])
                xk1 = xp.tile([P, P], DT.float32, tag="xk1")
                nc.sync.dma_start(out=xk1[:], in_=x_in[P:2 * P, t * P:(t + 1) * P])
                # node-major: psum[node, ch] = x_tile @ W1
                pn = psA.tile([P, HID], DT.float32, space="PSUM", tag="m1")
                nc.tensor.matmul(out=pn[:], lhsT=xk0[:], rhs=w1a[:],
                                 start=True, stop=False)
                nc.tensor.matmul(out=pn[:], lhsT=xk1[:], rhs=w1b[:],
                                 start=False, stop=True)
                ts_bf = wp.tile([P, HID], DT.bfloat16, tag="tsbf")
                nc.vector.tensor_scalar(
                    out=ts_bf[:], in0=pn[:], scalar1=dinv[:, t:t + 1],
                    scalar2=None, op0=mybir.AluOpType.mult)
                nc.sync.dma_start(out=ts_shard[t * P:(t + 1) * P, :], in_=ts_bf[:])

            if STOP == "B":
                nc.sync.dma_start(out=out_t[:, 0:P], in_=dinv_cols[0:OUT, 0:P])
            # ---------- phase C: allgather ts ----------
            if not STOP or STOP >= "C":
                nc.gpsimd.collective_compute(
                    "AllGather", mybir.AluOpType.bypass,
                    replica_groups=[list(range(NCORES))],
                    ins=[ts_shard[:]], outs=[ts_full[:]])

            # ---------- phase B2 (overlaps AG1): dinv_cols + tsT_own ----------
            for t in range(NT if (not STOP or STOP >= "B") else 0):
                diag = wp.tile([P, P], DT.float32, tag="diag")
                nc.vector.tensor_scalar(
                    out=diag[:], in0=identf[:], scalar1=dinv[:, t:t + 1],
                    scalar2=None, op0=mybir.AluOpType.mult)
                dps = psA.tile([P, P], DT.float32, space="PSUM", tag="m1")
                nc.tensor.matmul(out=dps[:], lhsT=ones[:], rhs=diag[:],
                                 start=True, stop=True)
                nc.vector.tensor_copy(out=dinv_cols[:, t * P:(t + 1) * P], in_=dps[:])
            for t in range(NT if (not STOP or STOP >= "B") else 0):
                xk0 = xp.tile([P, P], DT.float32, tag="xk0")
                nc.sync.dma_start(out=xk0[:], in_=x_in[0:P, t * P:(t + 1) * P])
                xk1 = xp.tile([P, P], DT.float32, tag="xk1")
                nc.sync.dma_start(out=xk1[:], in_=x_in[P:2 * P, t * P:(t + 1) * P])
                pt = psB.tile([HID, P], DT.float32, space="PSUM", tag="m2")
                nc.tensor.matmul(out=pt[:], lhsT=w1a[:], rhs=xk0[:],
                                 start=True, stop=False)
                nc.tensor.matmul(out=pt[:], lhsT=w1b[:], rhs=xk1[:],
                                 start=False, stop=True)
                nc.vector.tensor_tensor(
                    out=tsT_own[:, t * P:(t + 1) * P], in0=pt[:],
                    in1=dinv_cols[:, t * P:(t + 1) * P], op=mybir.AluOpType.mult)

            if STOP == "C":
                nc.sync.dma_start(out=out_t[:, 0:P], in_=dinv_cols[0:OUT, 0:P])
            # ---------- phase D/G: aggregation layers ----------
            def agg_layer(src_full, selfT, fch, bias, post, dst_sink):
                """fch: message channels used (HID or OUT); selfT [fch, NPAD];
                post(tile_idx, acc_sbuf_tile) emits per-tile outputs."""
                GW = 8   # chunks per gather instruction (<=1024 idxs)
                qrot = [0]
                tile_batches = []
                t0 = 0
                for bs in BATCH_SIZES:
                    tile_batches.append(list(range(t0, t0 + bs)))
                    t0 += bs
                for b, tiles in enumerate(tile_batches):
                    TBv = len(tiles)
                    CB_LOv = TBv * KLO
                    CB_HIv = TBv * KHI
                    M = mp.tile([P, CB * HID], DT.bfloat16, tag="M")
                    blo0 = tiles[0] * KLO
                    bhi0 = tiles[0] * KHI
                    for g0 in range(0, CB_LOv, GW):
                        w = min(GW, CB_LOv - g0)
                        n = w * P
                        col0 = (blo0 + g0) * 8
                        nc.gpsimd.dma_gather(
                            out_ap=M[:, g0 * HID:(g0 + w) * HID].rearrange(
                                "p (c f) -> p c f", f=HID),
                            in_ap=src_full[0:HALF, :],
                            idxs_ap=idx_lo[:, col0:col0 + w * 8],
                            num_idxs=n, num_idxs_reg=n, elem_size=HID,
                            queue_num=qrot[0] % 4)
                        qrot[0] += 1
                    for g0 in range(0, CB_HIv, GW):
                        w = min(GW, CB_HIv - g0)
                        n = w * P
                        col0 = (bhi0 + g0) * 8
                        nc.gpsimd.dma_gather(
                            out_ap=M[:, (CB_LOv + g0) * HID:(CB_LOv + g0 + w) * HID
                                     ].rearrange("p (c f) -> p c f", f=HID),
                            in_ap=src_full[HALF:NFULL, :],
                            idxs_ap=idx_hi[:, col0:col0 + w * 8],
                            num_idxs=n, num_idxs_reg=n, elem_size=HID,
                            queue_num=qrot[0] % 4)
                        qrot[0] += 1
                    for ti in range(TBv if SUB >= 2 else 0):
                        t = tiles[ti]
                        O = op.tile([P, KT * P], DT.bfloat16, tag="O")
                        ia = iota_bf[:]
                        iota_mid = bass.AP(
                            ia.tensor, ia.offset, [ia.ap[0], [0, KT], ia.ap[1]])
                        nc.vector.tensor_tensor(
                            out=O[:].rearrange("p (c j) -> p c j", j=P),
                            in0=dloc[:, t * KT:(t + 1) * KT].to_broadcast([P, KT, P]),
                            in1=iota_mid, op=mybir.AluOpType.is_equal)
                        if SUB < 3:
                            continue
                        acc = psC.tile([fch, P], DT.float32, space="PSUM", tag="acc")
                        for k in range(KLO):
                            c = (CB_LO - CB_LOv) // KLO * 0 + ti * KLO + k
                            nc.tensor.matmul(
                                out=acc[:],
                                lhsT=M[:, c * HID:c * HID + fch],
                                rhs=O[:, k * P:(k + 1) * P],
                                start=(k == 0), stop=False)
                        for k in range(KHI):
                            c = CB_LOv + ti * KHI + k
                            nc.tensor.matmul(
                                out=acc[:],
                                lhsT=M[:, c * HID:c * HID + fch],
                                rhs=O[:, (KLO + k) * P:(KLO + k + 1) * P],
                                start=False, stop=(k == KHI - 1))
                        # epilogue: (acc + selfT_t) * dinv_cols_t (+bias, act)
                        if SUB < 4:
                            z = wp.tile([fch, P], DT.float32, tag="a1")
                            nc.vector.tensor_copy(out=z[:], in_=acc[:])
                            continue
                        a1 = wp.tile([fch, P], DT.float32, tag="a1")
                        nc.vector.tensor_tensor(
                            out=a1[:], in0=acc[:],
                            in1=selfT[:, t * P:(t + 1) * P],
                            op=mybir.AluOpType.add)
                        nc.vector.tensor_tensor(
                            out=a1[:], in0=a1[:],
                            in1=dinv_cols[0:fch, t * P:(t + 1) * P],
                            op=mybir.AluOpType.mult)
                        post(t, a1, bias)

            def post_l1(t, a1, bias):
                h = wp.tile([HID, P], DT.float32, tag="h")
                nc.scalar.activation(h[:], a1[:],
                                     mybir.ActivationFunctionType.Relu,
                                     bias=bias[:], scale=1.0)
                nc.vector.tensor_tensor(
                    out=hsT[:, t * P:(t + 1) * P], in0=h[:],
                    in1=dinv_cols[:, t * P:(t + 1) * P],
                    op=mybir.AluOpType.mult)
                # GEMM2 for this tile immediately (feeds AG2 asap)
                p2 = psA.tile([P, OUT], DT.float32, space="PSUM", tag="m1")
                nc.tensor.matmul(out=p2[:], lhsT=hsT[:, t * P:(t + 1) * P],
                                 rhs=w2[:], start=True, stop=True)
                t2_bf = wp.tile([P, OUT], DT.bfloat16, tag="t2bf")
                nc.vector.tensor_copy(out=t2_bf[:], in_=p2[:])
                nc.sync.dma_start(out=t2_shard[t * P:(t + 1) * P, 0:OUT],
                                  in_=t2_bf[:])
                p2t = psB.tile([OUT, P], DT.float32, space="PSUM", tag="m2")
                nc.tensor.matmul(out=p2t[:], lhsT=w2[:],
                                 rhs=hsT[:, t * P:(t + 1) * P],
                                 start=True, stop=True)
                nc.vector.tensor_copy(out=t2T_own[:, t * P:(t + 1) * P], in_=p2t[:])

            if not STOP or STOP >= "D":
                agg_layer(ts_full, tsT_own, HID, b1t, post_l1, None)
            if STOP == "D":
                if SUB >= 4:
                    nc.gpsimd.dma_start(out=out_t[:, 0:P], in_=hsT[0:OUT, 0:P])
                else:
                    nc.sync.dma_start(out=out_t[:, 0:P], in_=dinv_cols[0:OUT, 0:P])

            if STOP == "E":
                nc.sync.dma_start(out=out_t[:, 0:P], in_=t2T_own[:, 0:P])
            # ---------- phase F: allgather t2 ----------
            if STOP and STOP < "F":
                pass
            else:
                nc.gpsimd.collective_compute(
                "AllGather", mybir.AluOpType.bypass,
                replica_groups=[list(range(NCORES))],
                ins=[t2_shard[:]], outs=[t2_full[:]])

            def post_l2(t, a1, bias):
                o = wp.tile([OUT, P], DT.float32, tag="o")
                nc.vector.tensor_scalar(
                    out=o[:], in0=a1[:], scalar1=bias[:], scalar2=None,
                    op0=mybir.AluOpType.add)
                nc.sync.dma_start(out=out_t[:, t * P:(t + 1) * P], in_=o[:])

            if not STOP:
                agg_layer(t2_full, t2T_own, OUT, b2t, post_l2, None)

    nc.compile()
    return nc


def kernel(x, edge_index, W1, b1, W2, b2):
    x = np.asarray(x, dtype=np.float32)
    W1 = np.asarray(W1, dtype=np.float32)
    b1 = np.asarray(b1, dtype=np.float32)
    W2 = np.asarray(W2, dtype=np.float32)
    b2 = np.asarray(b2, dtype=np.float32)

    import time as _t
    _t0 = _t.time()
    cores_dat, KLO, KHI, perm = _preprocess(edge_index)
    print(f"[kernel] preprocess {_t.time()-_t0:.1f}s KLO={KLO} KHI={KHI}", file=sys.stderr)
    _t0 = _t.time()
    nc = _build_program(KLO, KHI)
    print(f"[kernel] build+bacc-compile {_t.time()-_t0:.1f}s", file=sys.stderr)

    in_maps = []
    for c in range(NCORES):
        xt = np.zeros((IN_CH, NPAD), dtype=np.float32)
        xt[:, perm[c * NPC:(c + 1) * NPC]] = x[c * NPC:(c + 1) * NPC].T
        d = cores_dat[c]
        in_maps.append({
            "x_t": xt,
            "w1": W1,
            "b1": b1.reshape(HID, 1),
            "w2": W2,
            "b2": b2.reshape(OUT, 1),
            "deg": d["deg"],
            "idx_lo": d["idx_lo"],
            "idx_hi": d["idx_hi"],
            "dloc": d["dloc"],
        })
    _t0 = _t.time()
    trace = bool(os.environ.get("K_TRACE"))
    res = run_bass_kernel_spmd(nc, in_maps, list(range(NCORES)), trace=trace)
    print(f"[kernel] run {_t.time()-_t0:.1f}s", file=sys.stderr)
    global LAST_EXEC_NS, LAST_RES
    LAST_EXEC_NS = res.exec_time_ns
    LAST_RES = res
    if trace:
        print(f"[kernel] exec_time_ns={res.exec_time_ns}", file=sys.stderr)
    out = np.empty((N_NODES, OUT), dtype=np.float32)
    for c in range(NCORES):
        out[c * NPC:(c + 1) * NPC] = \
            res.results[c]["out_t"].T[perm[c * NPC:(c + 1) * NPC]]
    return out
